# revision 3
# baseline (speedup 1.0000x reference)
"""ChebNetConv (K=4) Bass kernel for 8 trn2 NeuronCores.

Strategy (1D row partitioning per sharding hint):
  - Nodes sharded across 8 cores (12500 rows each). Each SpMM step computes
    the core's own output rows; full neighbor tables (x / T1 / T2) are
    available to every core (x as replicated input; T1/T2 via AllGather).
  - SpMM core: edges grouped by (dest block of 128 rows, src chunk of 25000
    rows), padded to batches of 128.  Per batch: dma_gather pulls 128 source
    rows (256B bf16 each) into an SBUF tile G[128e, 128f]; a selector tile
    S[128e, 128d] (Laplacian values at (e, dest-in-block)) is built ON-CHIP
    by one fused DVE op from a compact (dst, val) stream:
        S = (iota_row == dst_col) * val_col
    PE matmul accumulates S.T @ G into the dest block's PSUM accumulator.
  - Chebyshev recurrence (T2 = 2*L@T1 - T0) folded into PSUM eviction.
  - Final linear: per dest block, PE-transpose cheb tiles to [f, n] and
    accumulate 4 matmuls against W slices + bias outer product.
"""

import itertools

import numpy as np

import concourse.bacc as bacc
import concourse.bass as bass
import concourse.mybir as mybir
import concourse.tile as tile
from concourse import bass_utils
from concourse.bass import ds
from concourse.masks import make_identity

P = 128


class Cfg:
    def __init__(self, n_nodes=100000, f=128, k=4, cores=8, chunk=25000, superb=8):
        assert n_nodes % cores == 0
        self.N = n_nodes
        self.F = f
        self.K = k
        self.CORES = cores
        self.RPC = n_nodes // cores            # rows per core
        self.NBLK = -(-self.RPC // P)          # dest blocks per core
        self.CHUNK = chunk                     # gather-table chunk rows (int16 idx limit)
        assert chunk <= 32767
        self.NCHUNK = -(-n_nodes // chunk)
        self.SUPER = superb                    # dest blocks per super-block
        self.NSUPER = -(-self.NBLK // superb)

    def blocks_of(self, s):
        return range(s * self.SUPER, min(self.NBLK, (s + 1) * self.SUPER))

    def nrows_of(self, b):
        return min(P, self.RPC - b * P)


def preprocess(cfg, rows, cols, vals):
    """Build per-core gather-index and (dst, val) selector streams.

    Returns (meta, per_core) where meta has compile-time batch counts
    (identical across cores) and per_core[c] = dict of input arrays.
    """
    rows = np.asarray(rows).astype(np.int64)
    cols = np.asarray(cols).astype(np.int64)
    vals = np.asarray(vals).astype(np.float32)

    core = rows // cfg.RPC
    loc = rows % cfg.RPC
    blk = loc // P
    dst = loc % P
    chk = cols // cfg.CHUNK
    src = (cols % cfg.CHUNK).astype(np.int64)

    counts = np.zeros((cfg.CORES, cfg.NBLK, cfg.NCHUNK), dtype=np.int64)
    np.add.at(counts, (core, blk, chk), 1)
    NB = np.maximum(1, -(-counts.max(axis=0) // P))  # [NBLK, NCHUNK] batches

    # slot (b, c) capacity NB[b,c]*128; slot start offsets in padded edge space,
    # ordered (super, chunk, block-in-super, batch)
    slot_start = np.zeros((cfg.NBLK, cfg.NCHUNK), dtype=np.int64)
    call_start = {}          # (s, c) -> padded-edge offset of the gather call
    call_nbatch = {}         # (s, c) -> total batches in call
    off = 0
    for s in range(cfg.NSUPER):
        for c in range(cfg.NCHUNK):
            call_start[(s, c)] = off
            nb = 0
            for b in cfg.blocks_of(s):
                slot_start[b, c] = off
                off += NB[b, c] * P
                nb += NB[b, c]
            call_nbatch[(s, c)] = nb
    tot_pad = off

    meta = dict(NB=NB, call_start=call_start, call_nbatch=call_nbatch,
                tot_pad=tot_pad)

    per_core = []
    for cid in range(cfg.CORES):
        m = core == cid
        key = (blk[m] * cfg.NCHUNK + chk[m])
        order = np.argsort(key, kind="stable")
        kb, kc, ksrc, kdst, kval = (blk[m][order], chk[m][order],
                                    src[m][order], dst[m][order],
                                    vals[m][order])
        # rank within slot
        cnt = counts[cid].reshape(-1)
        slot_flat = kb * cfg.NCHUNK + kc
        starts = np.zeros(cfg.NBLK * cfg.NCHUNK, dtype=np.int64)
        starts[1:] = np.cumsum(cnt)[:-1]
        rank = np.arange(len(kb)) - starts[slot_flat]
        pos = slot_start.reshape(-1)[slot_flat] + rank  # padded global position

        idx_flat = np.zeros(tot_pad, dtype=np.int16)
        idx_flat[pos] = ksrc.astype(np.int16)
        # compact selector stream: per padded edge (dst, val) f32 pairs
        dv = np.zeros((tot_pad, 2), dtype=np.float32)
        dv[pos, 0] = kdst.astype(np.float32)
        dv[pos, 1] = kval

        # idx DMA layout: per call, [128, 8*nb] with idx j at
        # [16g + j%16, j//16] for replica groups g=0..7
        idx_parts = []
        dv_parts = []
        for s in range(cfg.NSUPER):
            for c in range(cfg.NCHUNK):
                o = call_start[(s, c)]
                nb = call_nbatch[(s, c)]
                iv = idx_flat[o:o + nb * P]            # [nb*128]
                arr = iv.reshape(-1, 16).T             # [16, 8*nb]
                idx_parts.append(np.tile(arr, (8, 1)).reshape(-1))
                # dv layout per call: [128p, nb, 2]; edge at padded pos
                # p = pos%128 within batch q = pos//128 - call_start//128
                dvv = dv[o:o + nb * P].reshape(nb, P, 2)
                dv_parts.append(np.ascontiguousarray(
                    dvv.transpose(1, 0, 2)).reshape(-1))
        per_core.append(dict(
            idx_all=np.concatenate(idx_parts),
            dv_all=np.concatenate(dv_parts),
        ))
    return meta, per_core


def emulate(cfg, meta, per_core, tabs):
    """Numpy emulation of the on-device SpMM given gather tables per chunk.
    tabs: full [N, F] table. Returns per-core [RPC, F] segment sums."""
    NB = meta["NB"]
    outs = []
    for cid in range(cfg.CORES):
        pc = per_core[cid]
        out = np.zeros((cfg.RPC, cfg.F), dtype=np.float32)
        iofs = 0
        sofs = 0
        for s in range(cfg.NSUPER):
            for c in range(cfg.NCHUNK):
                nb = meta["call_nbatch"][(s, c)]
                w8 = nb * 8
                idx_tile = pc["idx_all"][iofs:iofs + 128 * w8].reshape(128, w8)
                iofs += 128 * w8
                n = nb * P
                unwrapped = idx_tile[:16, :].T.reshape(-1)[:n].astype(np.int64)
                g = tabs[c * cfg.CHUNK + unwrapped]      # [n, F]
                g = g.reshape(nb, P, cfg.F)
                dvt = pc["dv_all"][sofs:sofs + 128 * nb * 2].reshape(128, nb, 2)
                sofs += 128 * nb * 2
                q0 = 0
                for b in cfg.blocks_of(s):
                    for q in range(NB[b, c]):
                        dst = dvt[:, q0 + q, 0].astype(np.int64)
                        val = dvt[:, q0 + q, 1]
                        S = np.zeros((P, P), dtype=np.float32)
                        S[np.arange(P), dst] = val
                        G = g[q0 + q]                    # [128e, F]
                        out[b * P:b * P + cfg.nrows_of(b), :] += \
                            (S.T @ G)[:cfg.nrows_of(b)]
                    q0 += NB[b, c]
        outs.append(out)
    return outs


def build(cfg, meta):
    """Build the Bass program. Returns nc."""
    NB = meta["NB"]
    f32 = mybir.dt.float32
    bf16 = mybir.dt.bfloat16
    nc = bacc.Bacc("TRN2", target_bir_lowering=False, debug=False,
                   num_devices=cfg.CORES, num_swdge_queues=4)

    x_tab = nc.dram_tensor("x_tab", [cfg.N, cfg.F], bf16,
                           kind="ExternalInput")
    x_shard = nc.dram_tensor("x_shard", [cfg.RPC, cfg.F], f32,
                             kind="ExternalInput")
    idx_in = nc.dram_tensor("idx_all", [len_idx(cfg, meta)], mybir.dt.int16,
                            kind="ExternalInput")
    dv_in = nc.dram_tensor("dv_all", [meta["tot_pad"] * 2], f32,
                           kind="ExternalInput")
    w_in = nc.dram_tensor("w_lhsT", [cfg.F, cfg.K * cfg.F], f32,
                          kind="ExternalInput")
    b_in = nc.dram_tensor("b_row", [1, cfg.F], f32, kind="ExternalInput")
    out_shard = nc.dram_tensor("out_shard", [cfg.RPC, cfg.F], f32,
                               kind="ExternalOutput")

    rg = [list(range(cfg.CORES))]
    qrr = itertools.count()  # global gather-queue round robin

    with tile.TileContext(nc) as tc:
        with tc.tile_pool(name="dram", bufs=1, space="DRAM") as dram:
            t1_shard = dram.tile([cfg.RPC, cfg.F], f32, tag="t1s")
            t2_shard = dram.tile([cfg.RPC, cfg.F], f32, tag="t2s")
            t3_shard = dram.tile([cfg.RPC, cfg.F], f32, tag="t3s")
            t1_tab = dram.tile([cfg.N, cfg.F], bf16, tag="t1t",
                               addr_space="Shared")
            t2_tab = dram.tile([cfg.N, cfg.F], bf16, tag="t2t",
                               addr_space="Shared")
            t1_bsh = dram.tile([cfg.RPC, cfg.F], bf16, tag="t1b")
            t2_bsh = dram.tile([cfg.RPC, cfg.F], bf16, tag="t2b")

            with (
                tc.tile_pool(name="const", bufs=1) as constp,
                tc.tile_pool(name="gpool", bufs=2) as gpool,
                tc.tile_pool(name="spool", bufs=6) as spool,
                tc.tile_pool(name="ipool", bufs=2) as ipool,
                tc.tile_pool(name="dvpool", bufs=2) as dvpool,
                tc.tile_pool(name="psum", bufs=2, space="PSUM") as pspool,
                tc.tile_pool(name="ev", bufs=4) as evpool,
            ):
                iota_row = constp.tile([P, P], bf16)
                nc.gpsimd.iota(iota_row[:], pattern=[[1, P]], base=0,
                               channel_multiplier=0,
                               allow_small_or_imprecise_dtypes=True)
                for step in (1, 2, 3):
                    src = {1: x_tab[:], 2: t1_tab[:], 3: t2_tab[:]}[step]
                    prev = {1: None, 2: x_shard, 3: t1_shard}[step]
                    dst = {1: t1_shard, 2: t2_shard, 3: t3_shard}[step]
                    bdst = {1: t1_bsh, 2: t2_bsh, 3: None}[step]
                    spmm_step(cfg, meta, nc, tc, gpool, spool, ipool, dvpool,
                              pspool, evpool, idx_in, dv_in, iota_row, src,
                              prev, dst, step, qrr, bdst)
                    if step == 1:
                        nc.gpsimd.collective_compute(
                            "AllGather", mybir.AluOpType.bypass,
                            replica_groups=rg, ins=[t1_bsh[:].opt()],
                            outs=[t1_tab[:].opt()])
                    elif step == 2:
                        nc.gpsimd.collective_compute(
                            "AllGather", mybir.AluOpType.bypass,
                            replica_groups=rg, ins=[t2_bsh[:].opt()],
                            outs=[t2_tab[:].opt()])

            with (
                tc.tile_pool(name="fconst", bufs=1) as fconst,
                tc.tile_pool(name="fload", bufs=3) as fload,
                tc.tile_pool(name="ftrans", bufs=3) as ftrans,
                tc.tile_pool(name="fpsum", bufs=2, space="PSUM") as fpsum,
                tc.tile_pool(name="fout", bufs=3) as foutp,
            ):
                ident = fconst.tile([P, P], f32)
                make_identity(nc, ident[:])
                wt = fconst.tile([cfg.F, cfg.K, cfg.F], f32)
                nc.sync.dma_start(wt[:], w_in[:].rearrange(
                    "f (k o) -> f k o", k=cfg.K))
                brow = fconst.tile([1, cfg.F], f32)
                nc.sync.dma_start(brow[:], b_in[:])
                ones = fconst.tile([1, P], f32)
                nc.vector.memset(ones[:], 1.0)

                shards = [x_shard, t1_shard, t2_shard, t3_shard]
                for b in range(cfg.NBLK):
                    nrows = cfg.nrows_of(b)
                    r0 = b * P
                    opsum = fpsum.tile([P, cfg.F], f32, tag="opsum")
                    for k in range(cfg.K):
                        ct = fload.tile([P, cfg.F], f32, tag="cheb")
                        sh = shards[k]
                        nc.sync.dma_start(ct[:nrows, :],
                                          sh[r0:r0 + nrows, :])
                        tp = fpsum.tile([P, P], f32, tag="tpsum")
                        nc.tensor.transpose(tp[:, :nrows], ct[:nrows, :],
                                            ident[:nrows, :nrows])
                        cT = ftrans.tile([cfg.F, P], f32, tag="chebT")
                        nc.vector.tensor_copy(cT[:, :nrows], tp[:, :nrows])
                        nc.tensor.matmul(opsum[:nrows, :], cT[:, :nrows],
                                         wt[:, k, :], start=(k == 0),
                                         stop=False)
                    nc.tensor.matmul(opsum[:nrows, :], ones[:1, :nrows],
                                     brow[:1, :], start=False, stop=True)
                    ot = foutp.tile([P, cfg.F], f32, tag="ot")
                    nc.vector.tensor_copy(ot[:nrows, :], opsum[:nrows, :])
                    nc.scalar.dma_start(out_shard[r0:r0 + nrows, :],
                                        ot[:nrows, :])

    nc.compile()
    return nc


def len_idx(cfg, meta):
    return meta["tot_pad"] * 8  # 128 parts * 8*nb cols per call of nb*128 idxs


def spmm_step(cfg, meta, nc, tc, gpool, spool, ipool, dvpool, pspool, evpool,
              idx_in, dv_in, iota_row, src, prev, dst, step, qrr, bdst=None):
    NB = meta["NB"]
    f32 = mybir.dt.float32
    bf16 = mybir.dt.bfloat16
    sub = mybir.AluOpType.subtract
    iofs = 0
    sofs = 0
    for s in range(cfg.NSUPER):
        blocks = list(cfg.blocks_of(s))
        ps = [pspool.tile([P, 4, cfg.F], f32, tag=f"ps{i}", name=f"ps{i}")
              for i in range(-(-len(blocks) // 4))]
        for c in range(cfg.NCHUNK):
            nb = meta["call_nbatch"][(s, c)]
            w8 = nb * 8
            ix = ipool.tile([P, w8], mybir.dt.int16, tag="ix")
            nc.sync.dma_start(
                ix[:], idx_in[iofs:iofs + P * w8].rearrange(
                    "(p w) -> p w", p=P))
            iofs += P * w8
            dv = dvpool.tile([P, nb, 2], f32, tag="dv")
            nc.sync.dma_start(
                dv[:], dv_in[sofs:sofs + P * nb * 2].rearrange(
                    "(p b t) -> p b t", p=P, b=nb))
            sofs += P * nb * 2
            g = gpool.tile([P, nb, cfg.F], bf16, tag="G")
            lo = c * cfg.CHUNK
            hi = min(cfg.N, lo + cfg.CHUNK)
            # split into sub-calls: 8 batches = 1024 idxs keeps each SDMA
            # engine's packet at the 64-descriptor single-packet limit
            MAXB = 8
            for b0 in range(0, nb, MAXB):
                b1 = min(nb, b0 + MAXB)
                nc.gpsimd.dma_gather(
                    g[:, b0:b1, :], src[lo:hi, :],
                    ix[:, b0 * 8:b1 * 8], (b1 - b0) * P, (b1 - b0) * P,
                    cfg.F, queue_num=next(qrr) % 4)
            q0 = 0
            for bi, b in enumerate(blocks):
                pt = ps[bi // 4][:, bi % 4, :]
                # one accumulation group per PSUM bank: start clears
                # has_written bank-wide, so only the first matmul into the
                # bank may set it; per-element has_written handles the
                # disjoint block slices.
                last_in_bank = bi % 4 == 3 or bi == len(blocks) - 1
                for q in range(NB[b, c]):
                    sl = spool.tile([P, P], bf16, tag="S")
                    nc.vector.tensor_scalar(
                        sl[:], iota_row[:], dv[:, q0 + q, 0:1],
                        dv[:, q0 + q, 1:2], mybir.AluOpType.is_equal,
                        mybir.AluOpType.mult)
                    nc.tensor.matmul(
                        pt, sl[:], g[:, q0 + q, :],
                        start=(c == 0 and q == 0 and bi % 4 == 0),
                        stop=(c == cfg.NCHUNK - 1 and q == NB[b, c] - 1
                              and last_in_bank),
                        skip_group_check=True)
                q0 += NB[b, c]
        for bi, b in enumerate(blocks):
            pt = ps[bi // 4][:, bi % 4, :]
            nrows = cfg.nrows_of(b)
            r0 = b * P
            ev = evpool.tile([P, cfg.F], f32, tag="ev")
            if prev is None:
                nc.vector.tensor_copy(ev[:nrows, :], pt[:nrows, :])
            else:
                pv = evpool.tile([P, cfg.F], f32, tag="pv")
                nc.sync.dma_start(pv[:nrows, :], prev[r0:r0 + nrows, :])
                nc.vector.tensor_scalar_mul(ev[:nrows, :], pt[:nrows, :], 2.0)
                nc.vector.tensor_tensor(ev[:nrows, :], ev[:nrows, :],
                                        pv[:nrows, :], op=sub)
            nc.scalar.dma_start(dst[r0:r0 + nrows, :], ev[:nrows, :])
            if bdst is not None:
                evb = evpool.tile([P, cfg.F], bf16, tag="evb")
                nc.vector.tensor_copy(evb[:nrows, :], ev[:nrows, :])
                nc.scalar.dma_start(bdst[r0:r0 + nrows, :], evb[:nrows, :])


def make_inputs(cfg, meta, per_core, x, W, b):
    import ml_dtypes
    x = np.asarray(x, dtype=np.float32)
    W = np.asarray(W, dtype=np.float32)
    b = np.asarray(b, dtype=np.float32)
    # w_lhsT[f, k, o] = W[o, f*K + k]
    wl = W.reshape(cfg.F, cfg.F, cfg.K).transpose(1, 2, 0)  # W[o, f, k] -> [f,k,o]
    wl = np.ascontiguousarray(wl).reshape(cfg.F, cfg.K * cfg.F)
    x_tab = x.astype(ml_dtypes.bfloat16)
    in_maps = []
    for cid in range(cfg.CORES):
        in_maps.append({
            "x_tab": x_tab,
            "x_shard": np.ascontiguousarray(
                x[cid * cfg.RPC:(cid + 1) * cfg.RPC]),
            "idx_all": per_core[cid]["idx_all"],
            "dv_all": per_core[cid]["dv_all"],
            "w_lhsT": wl,
            "b_row": b.reshape(1, cfg.F),
        })
    return in_maps


def kernel(x, lap_rows, lap_cols, lap_vals, W, b, k):
    cfg = Cfg()
    assert int(k) == cfg.K
    meta, per_core = preprocess(cfg, lap_rows, lap_cols, lap_vals)
    nc = build(cfg, meta)
    in_maps = make_inputs(cfg, meta, per_core, x, W, b)
    res = bass_utils.run_bass_kernel_spmd(
        nc, in_maps, core_ids=list(range(cfg.CORES)))
    out = np.concatenate([res.results[c]["out_shard"]
                          for c in range(cfg.CORES)], axis=0)
    return out.astype(np.float32)


# revision 8
# speedup vs baseline: 1.2004x; 1.2004x over previous
"""ChebNetConv (K=4) Bass kernel for 8 trn2 NeuronCores.

Strategy (1D row partitioning per sharding hint):
  - Nodes sharded across 8 cores (12500 rows each). Each SpMM step computes
    the core's own output rows; full neighbor tables (x / T1 / T2) are
    available to every core (x as replicated input; T1/T2 via AllGather).
  - SpMM core: edges grouped by (dest block of 128 rows, src chunk) and
    padded to batches of 128.  Per (superblock, chunk) call: dma_gather
    pulls the call's source rows (256B bf16 each) into an SBUF tile
    G[128e, nb, 128f]; selector tiles S[128e, nb, 128d] (Laplacian values
    at (e, dest-in-block)) are built ON-CHIP by two batched DVE passes
    from a compact (dst, val) stream:
        S = is_equal(iota_row, dst_bcast) * val_bcast
    PE matmuls accumulate S[:,q,:].T @ G[:,q,:] into dest-block PSUM.
  - Chebyshev recurrence (T2 = 2*L@T1 - T0) fused into batched PSUM
    eviction (one scalar_tensor_tensor per 4-block PSUM bank).
  - Neighbor tables are split into two halves (A = dest blocks 0..47,
    B = 48..97 of every rank) with separate AllGathers, so the first AG
    overlaps the second half of the producing step and the next step
    starts on half-A chunks while AG-B completes.
  - Final linear: per dest block, PE-transpose cheb tiles to [f, n] and
    accumulate 4 matmuls against W slices + bias outer product.
"""

import itertools

import numpy as np

import concourse.bacc as bacc
import concourse.bass as bass
import concourse.mybir as mybir
import concourse.tile as tile
from concourse import bass_utils
from concourse.bass import ds
from concourse.masks import make_identity

P = 128


class Cfg:
    def __init__(self, n_nodes=100000, f=128, k=4, cores=8, superb=8,
                 half_blocks=48):
        assert n_nodes % cores == 0
        self.N = n_nodes
        self.F = f
        self.K = k
        self.CORES = cores
        self.RPC = n_nodes // cores            # rows per core
        self.NBLK = -(-self.RPC // P)          # dest blocks per core
        self.SUPER = superb                    # dest blocks per super-block
        self.NSUPER = -(-self.NBLK // superb)
        # half split for pipelined AllGathers (block-aligned)
        self.HBLK = half_blocks                # blocks in half A
        self.ROWS_A = half_blocks * P          # 6144 local rows in half A
        self.ROWS_B = self.RPC - self.ROWS_A   # 6356 local rows in half B
        self.NA = cores * self.ROWS_A          # tabA rows (49152)
        self.NBR = cores * self.ROWS_B         # tabB rows (50848)
        self.CA = self.NA // 2                 # chunk size within A (24576)
        self.CB = self.NBR // 2                # chunk size within B (25424)
        assert self.NA % 2 == 0 and self.NBR % 2 == 0
        assert self.CA <= 32767 and self.CB <= 32767  # int16 idx limit
        self.NCHUNK = 4
        # chunk -> (table, lo, hi); table 0 = A, 1 = B
        self.chunk_spec = [(0, 0, self.CA), (0, self.CA, 2 * self.CA),
                           (1, 0, self.CB), (1, self.CB, 2 * self.CB)]
        self.AG_SUPER = half_blocks // superb - 1  # super after which half A done

    def blocks_of(self, s):
        return range(s * self.SUPER, min(self.NBLK, (s + 1) * self.SUPER))

    def nrows_of(self, b):
        return min(P, self.RPC - b * P)

    def map_cols(self, cols):
        """Map original node ids -> (chunk, idx-within-chunk) in the
        permuted [A;B] table layout."""
        o = cols // self.RPC
        loc = cols % self.RPC
        in_a = loc < self.ROWS_A
        row_a = o * self.ROWS_A + loc
        row_b = o * self.ROWS_B + (loc - self.ROWS_A)
        chk = np.where(in_a, row_a // self.CA, 2 + row_b // self.CB)
        src = np.where(in_a, row_a % self.CA, row_b % self.CB)
        return chk.astype(np.int64), src.astype(np.int64)

    def perm_halves(self, full):
        """Split a [N, F] array into the permuted tabA/tabB layouts."""
        a = np.concatenate([full[o * self.RPC:o * self.RPC + self.ROWS_A]
                            for o in range(self.CORES)], axis=0)
        b = np.concatenate([full[o * self.RPC + self.ROWS_A:(o + 1) * self.RPC]
                            for o in range(self.CORES)], axis=0)
        return a, b


def preprocess(cfg, rows, cols, vals):
    """Build per-core gather-index and (dst, val) selector streams.

    Returns (meta, per_core) where meta has compile-time batch counts
    (identical across cores) and per_core[c] = dict of input arrays.
    """
    rows = np.asarray(rows).astype(np.int64)
    cols = np.asarray(cols).astype(np.int64)
    vals = np.asarray(vals).astype(np.float32)

    core = rows // cfg.RPC
    loc = rows % cfg.RPC
    blk = loc // P
    dst = loc % P
    chk, src = cfg.map_cols(cols)

    counts = np.zeros((cfg.CORES, cfg.NBLK, cfg.NCHUNK), dtype=np.int64)
    np.add.at(counts, (core, blk, chk), 1)
    NB = np.maximum(1, -(-counts.max(axis=0) // P))  # [NBLK, NCHUNK] batches

    # slot (b, c) capacity NB[b,c]*128; slot start offsets in padded edge space,
    # ordered (super, chunk, block-in-super, batch)
    slot_start = np.zeros((cfg.NBLK, cfg.NCHUNK), dtype=np.int64)
    call_start = {}          # (s, c) -> padded-edge offset of the gather call
    call_nbatch = {}         # (s, c) -> total batches in call
    off = 0
    for s in range(cfg.NSUPER):
        for c in range(cfg.NCHUNK):
            call_start[(s, c)] = off
            nb = 0
            for b in cfg.blocks_of(s):
                slot_start[b, c] = off
                off += NB[b, c] * P
                nb += NB[b, c]
            call_nbatch[(s, c)] = nb
    tot_pad = off

    meta = dict(NB=NB, call_start=call_start, call_nbatch=call_nbatch,
                tot_pad=tot_pad)

    import ml_dtypes
    per_core = []
    for cid in range(cfg.CORES):
        m = core == cid
        key = (blk[m] * cfg.NCHUNK + chk[m])
        order = np.argsort(key, kind="stable")
        kb, kc, ksrc, kdst, kval = (blk[m][order], chk[m][order],
                                    src[m][order], dst[m][order],
                                    vals[m][order])
        # rank within slot
        cnt = counts[cid].reshape(-1)
        slot_flat = kb * cfg.NCHUNK + kc
        starts = np.zeros(cfg.NBLK * cfg.NCHUNK, dtype=np.int64)
        starts[1:] = np.cumsum(cnt)[:-1]
        rank = np.arange(len(kb)) - starts[slot_flat]
        pos = slot_start.reshape(-1)[slot_flat] + rank  # padded global position

        idx_flat = np.zeros(tot_pad, dtype=np.int16)
        idx_flat[pos] = ksrc.astype(np.int16)
        # compact selector stream: per padded edge (dst, val) bf16 pairs
        dv = np.zeros((tot_pad, 2), dtype=ml_dtypes.bfloat16)
        dv[pos, 0] = kdst.astype(ml_dtypes.bfloat16)
        dv[pos, 1] = kval.astype(ml_dtypes.bfloat16)

        # idx DMA layout: per call, [128, 8*nb] with idx j at
        # [16g + j%16, j//16] for replica groups g=0..7
        idx_parts = []
        dv_parts = []
        for s in range(cfg.NSUPER):
            for c in range(cfg.NCHUNK):
                o = call_start[(s, c)]
                nb = call_nbatch[(s, c)]
                iv = idx_flat[o:o + nb * P]            # [nb*128]
                arr = iv.reshape(-1, 16).T             # [16, 8*nb]
                idx_parts.append(np.tile(arr, (8, 1)).reshape(-1))
                # dv layout per call: [128p, nb, 2]
                dvv = dv[o:o + nb * P].reshape(nb, P, 2)
                dv_parts.append(np.ascontiguousarray(
                    dvv.transpose(1, 0, 2)).reshape(-1))
        per_core.append(dict(
            idx_all=np.concatenate(idx_parts),
            dv_all=np.concatenate(dv_parts),
        ))
    return meta, per_core


def emulate(cfg, meta, per_core, full_tab):
    """Numpy emulation of the on-device SpMM. full_tab: [N, F] table in
    ORIGINAL node order. Returns per-core [RPC, F] segment sums."""
    NB = meta["NB"]
    ta, tb = cfg.perm_halves(full_tab)
    chunk_tabs = [ta[:cfg.CA], ta[cfg.CA:], tb[:cfg.CB], tb[cfg.CB:]]
    outs = []
    for cid in range(cfg.CORES):
        pc = per_core[cid]
        out = np.zeros((cfg.RPC, cfg.F), dtype=np.float32)
        iofs = 0
        sofs = 0
        for s in range(cfg.NSUPER):
            for c in range(cfg.NCHUNK):
                nb = meta["call_nbatch"][(s, c)]
                w8 = nb * 8
                idx_tile = pc["idx_all"][iofs:iofs + 128 * w8].reshape(128, w8)
                iofs += 128 * w8
                n = nb * P
                unwrapped = idx_tile[:16, :].T.reshape(-1)[:n].astype(np.int64)
                g = chunk_tabs[c][unwrapped]             # [n, F]
                g = g.reshape(nb, P, cfg.F)
                dvt = pc["dv_all"][sofs:sofs + 128 * nb * 2].reshape(128, nb, 2)
                sofs += 128 * nb * 2
                q0 = 0
                for b in cfg.blocks_of(s):
                    for q in range(NB[b, c]):
                        dd = dvt[:, q0 + q, 0].astype(np.int64)
                        vv = dvt[:, q0 + q, 1].astype(np.float32)
                        S = np.zeros((P, P), dtype=np.float32)
                        S[np.arange(P), dd] = vv
                        G = g[q0 + q].astype(np.float32)
                        out[b * P:b * P + cfg.nrows_of(b), :] += \
                            (S.T @ G)[:cfg.nrows_of(b)]
                    q0 += NB[b, c]
        outs.append(out)
    return outs


def build(cfg, meta):
    """Build the Bass program. Returns nc."""
    f32 = mybir.dt.float32
    bf16 = mybir.dt.bfloat16
    nc = bacc.Bacc("TRN2", target_bir_lowering=False, debug=False,
                   num_devices=cfg.CORES, num_swdge_queues=4)

    x_tabA = nc.dram_tensor("x_tabA", [cfg.NA, cfg.F], bf16,
                            kind="ExternalInput")
    x_tabB = nc.dram_tensor("x_tabB", [cfg.NBR, cfg.F], bf16,
                            kind="ExternalInput")
    x_shard = nc.dram_tensor("x_shard", [cfg.RPC, cfg.F], f32,
                             kind="ExternalInput")
    idx_in = nc.dram_tensor("idx_all", [meta["tot_pad"] * 8], mybir.dt.int16,
                            kind="ExternalInput")
    dv_in = nc.dram_tensor("dv_all", [meta["tot_pad"] * 2], bf16,
                           kind="ExternalInput")
    w_in = nc.dram_tensor("w_lhsT", [cfg.F, cfg.K * cfg.F], f32,
                          kind="ExternalInput")
    b_in = nc.dram_tensor("b_row", [1, cfg.F], f32, kind="ExternalInput")
    out_shard = nc.dram_tensor("out_shard", [cfg.RPC, cfg.F], f32,
                               kind="ExternalOutput")

    rg = [list(range(cfg.CORES))]
    qrr = itertools.count()  # global gather-queue round robin

    with tile.TileContext(nc) as tc:
        with tc.tile_pool(name="dram", bufs=1, space="DRAM") as dram:
            t1_shard = dram.tile([cfg.RPC, cfg.F], f32, tag="t1s")
            t2_shard = dram.tile([cfg.RPC, cfg.F], f32, tag="t2s")
            t3_shard = dram.tile([cfg.RPC, cfg.F], f32, tag="t3s")
            t1_bshA = dram.tile([cfg.ROWS_A, cfg.F], bf16, tag="t1bA")
            t1_bshB = dram.tile([cfg.ROWS_B, cfg.F], bf16, tag="t1bB")
            t2_bshA = dram.tile([cfg.ROWS_A, cfg.F], bf16, tag="t2bA")
            t2_bshB = dram.tile([cfg.ROWS_B, cfg.F], bf16, tag="t2bB")
            t1_tabA = dram.tile([cfg.NA, cfg.F], bf16, tag="t1tA",
                                addr_space="Shared")
            t1_tabB = dram.tile([cfg.NBR, cfg.F], bf16, tag="t1tB",
                                addr_space="Shared")
            t2_tabA = dram.tile([cfg.NA, cfg.F], bf16, tag="t2tA",
                                addr_space="Shared")
            t2_tabB = dram.tile([cfg.NBR, cfg.F], bf16, tag="t2tB",
                                addr_space="Shared")

            def ag(bsh, tab):
                nc.gpsimd.collective_compute(
                    "AllGather", mybir.AluOpType.bypass, replica_groups=rg,
                    ins=[bsh[:].opt()], outs=[tab[:].opt()])

            with (
                tc.tile_pool(name="const", bufs=1) as constp,
                tc.tile_pool(name="gpool", bufs=3) as gpool,
                tc.tile_pool(name="spool", bufs=2) as spool,
                tc.tile_pool(name="ipool", bufs=2) as ipool,
                tc.tile_pool(name="dvpool", bufs=2) as dvpool,
                tc.tile_pool(name="psum", bufs=2, space="PSUM") as pspool,
                tc.tile_pool(name="ev", bufs=3) as evpool,
            ):
                iota_row = constp.tile([P, P], bf16)
                nc.gpsimd.iota(iota_row[:], pattern=[[1, P]], base=0,
                               channel_multiplier=0,
                               allow_small_or_imprecise_dtypes=True)
                for step in (1, 2, 3):
                    tabs = {1: (x_tabA[:], x_tabB[:]),
                            2: (t1_tabA[:], t1_tabB[:]),
                            3: (t2_tabA[:], t2_tabB[:])}[step]
                    prev = {1: None, 2: x_shard, 3: t1_shard}[step]
                    dst = {1: t1_shard, 2: t2_shard, 3: t3_shard}[step]
                    bA, bB = {1: (t1_bshA, t1_bshB), 2: (t2_bshA, t2_bshB),
                              3: (None, None)}[step]
                    hooks = {}
                    if step == 1:
                        hooks = {cfg.AG_SUPER: lambda: ag(t1_bshA, t1_tabA),
                                 cfg.NSUPER - 1: lambda: ag(t1_bshB, t1_tabB)}
                    elif step == 2:
                        hooks = {cfg.AG_SUPER: lambda: ag(t2_bshA, t2_tabA),
                                 cfg.NSUPER - 1: lambda: ag(t2_bshB, t2_tabB)}
                    spmm_step(cfg, meta, nc, tc, gpool, spool, ipool, dvpool,
                              pspool, evpool, idx_in, dv_in, iota_row, tabs,
                              prev, dst, qrr, bA, bB, hooks)

            with (
                tc.tile_pool(name="fconst", bufs=1) as fconst,
                tc.tile_pool(name="fload", bufs=3) as fload,
                tc.tile_pool(name="ftrans", bufs=3) as ftrans,
                tc.tile_pool(name="fpsum", bufs=2, space="PSUM") as fpsum,
                tc.tile_pool(name="fout", bufs=3) as foutp,
            ):
                ident = fconst.tile([P, P], f32)
                make_identity(nc, ident[:])
                wt = fconst.tile([cfg.F, cfg.K, cfg.F], f32)
                nc.sync.dma_start(wt[:], w_in[:].rearrange(
                    "f (k o) -> f k o", k=cfg.K))
                brow = fconst.tile([1, cfg.F], f32)
                nc.sync.dma_start(brow[:], b_in[:])
                ones = fconst.tile([1, P], f32)
                nc.vector.memset(ones[:], 1.0)

                shards = [x_shard, t1_shard, t2_shard, t3_shard]
                for b in range(cfg.NBLK):
                    nrows = cfg.nrows_of(b)
                    r0 = b * P
                    opsum = fpsum.tile([P, cfg.F], f32, tag="opsum")
                    for k in range(cfg.K):
                        ct = fload.tile([P, cfg.F], f32, tag="cheb")
                        sh = shards[k]
                        nc.sync.dma_start(ct[:nrows, :],
                                          sh[r0:r0 + nrows, :])
                        tp = fpsum.tile([P, P], f32, tag="tpsum")
                        nc.tensor.transpose(tp[:, :nrows], ct[:nrows, :],
                                            ident[:nrows, :nrows])
                        cT = ftrans.tile([cfg.F, P], f32, tag="chebT")
                        nc.vector.tensor_copy(cT[:, :nrows], tp[:, :nrows])
                        nc.tensor.matmul(opsum[:nrows, :], cT[:, :nrows],
                                         wt[:, k, :], start=(k == 0),
                                         stop=False)
                    nc.tensor.matmul(opsum[:nrows, :], ones[:1, :nrows],
                                     brow[:1, :], start=False, stop=True)
                    ot = foutp.tile([P, cfg.F], f32, tag="ot")
                    nc.vector.tensor_copy(ot[:nrows, :], opsum[:nrows, :])
                    nc.scalar.dma_start(out_shard[r0:r0 + nrows, :],
                                        ot[:nrows, :])

    nc.compile()
    return nc


def spmm_step(cfg, meta, nc, tc, gpool, spool, ipool, dvpool, pspool, evpool,
              idx_in, dv_in, iota_row, tabs, prev, dst, qrr, bA, bB, hooks):
    NB = meta["NB"]
    f32 = mybir.dt.float32
    bf16 = mybir.dt.bfloat16
    eq = mybir.AluOpType.is_equal
    mul = mybir.AluOpType.mult
    sub = mybir.AluOpType.subtract
    iofs = 0
    sofs = 0
    for s in range(cfg.NSUPER):
        blocks = list(cfg.blocks_of(s))
        ps = [pspool.tile([P, 4, cfg.F], f32, tag=f"ps{i}", name=f"ps{i}")
              for i in range(-(-len(blocks) // 4))]
        for c in range(cfg.NCHUNK):
            tab_i, lo, hi = cfg.chunk_spec[c]
            src = tabs[tab_i]
            nb = meta["call_nbatch"][(s, c)]
            w8 = nb * 8
            ix = ipool.tile([P, w8], mybir.dt.int16, tag="ix")
            nc.sync.dma_start(
                ix[:], idx_in[iofs:iofs + P * w8].rearrange(
                    "(p w) -> p w", p=P))
            iofs += P * w8
            dv = dvpool.tile([P, nb, 2], bf16, tag="dv")
            nc.sync.dma_start(
                dv[:], dv_in[sofs:sofs + P * nb * 2].rearrange(
                    "(p b t) -> p b t", p=P, b=nb))
            sofs += P * nb * 2
            g = gpool.tile([P, nb, cfg.F], bf16, tag="G")
            # split into sub-calls: 8 batches = 1024 idxs keeps each SDMA
            # engine's packet at the 64-descriptor single-packet limit
            MAXB = 8
            for b0 in range(0, nb, MAXB):
                b1 = min(nb, b0 + MAXB)
                nc.gpsimd.dma_gather(
                    g[:, b0:b1, :], src[lo:hi, :],
                    ix[:, b0 * 8:b1 * 8], (b1 - b0) * P, (b1 - b0) * P,
                    cfg.F, queue_num=next(qrr) % 4)
            # batched on-chip selector build (2 DVE passes over the call)
            sl = spool.tile([P, nb, P], bf16, tag="S")
            iota_b = iota_row[:].unsqueeze(1).broadcast_to([P, nb, P])
            nc.vector.tensor_tensor(
                sl[:], iota_b, dv[:, :, 0:1].broadcast_to([P, nb, P]), op=eq)
            nc.vector.tensor_tensor(
                sl[:], sl[:], dv[:, :, 1:2].broadcast_to([P, nb, P]), op=mul)
            q0 = 0
            for bi, b in enumerate(blocks):
                pt = ps[bi // 4][:, bi % 4, :]
                # one accumulation group per PSUM bank: start clears
                # has_written bank-wide, so only the first matmul into the
                # bank may set it; per-element has_written handles the
                # disjoint block slices.
                last_in_bank = bi % 4 == 3 or bi == len(blocks) - 1
                for q in range(NB[b, c]):
                    nc.tensor.matmul(
                        pt, sl[:, q0 + q, :], g[:, q0 + q, :],
                        start=(c == 0 and q == 0 and bi % 4 == 0),
                        stop=(c == cfg.NCHUNK - 1 and q == NB[b, c] - 1
                              and last_in_bank),
                        skip_group_check=True)
                q0 += NB[b, c]
        # eviction: batched per 4-block PSUM bank where possible
        gi = 0
        while gi < len(blocks):
            grp = blocks[gi:gi + 4]
            pst = ps[gi // 4]
            full = len(grp) == 4 and all(cfg.nrows_of(b) == P for b in grp)
            if full:
                b0 = grp[0]
                r0 = b0 * P
                ev4 = evpool.tile([P, 4, cfg.F], f32, tag="ev4")
                if prev is None:
                    nc.vector.tensor_copy(ev4[:], pst[:])
                else:
                    pv4 = evpool.tile([P, 4, cfg.F], f32, tag="pv4")
                    nc.sync.dma_start(pv4[:], prev[r0:r0 + 4 * P, :].rearrange(
                        "(g p) f -> p g f", p=P))
                    nc.vector.scalar_tensor_tensor(
                        ev4[:], pst[:], 2.0, pv4[:], op0=mul, op1=sub)
                nc.scalar.dma_start(
                    dst[r0:r0 + 4 * P, :].rearrange("(g p) f -> p g f", p=P),
                    ev4[:])
                if bA is not None:
                    evb = evpool.tile([P, 4, cfg.F], bf16, tag="evb4")
                    nc.vector.tensor_copy(evb[:], ev4[:])
                    if b0 < cfg.HBLK:
                        tgt, tr0 = bA, b0 * P
                    else:
                        tgt, tr0 = bB, (b0 - cfg.HBLK) * P
                    nc.scalar.dma_start(
                        tgt[tr0:tr0 + 4 * P, :].rearrange(
                            "(g p) f -> p g f", p=P), evb[:])
                gi += 4
            else:
                for j, b in enumerate(grp):
                    pt = pst[:, j, :]
                    nrows = cfg.nrows_of(b)
                    r0 = b * P
                    ev = evpool.tile([P, cfg.F], f32, tag="ev")
                    if prev is None:
                        nc.vector.tensor_copy(ev[:nrows, :], pt[:nrows, :])
                    else:
                        pv = evpool.tile([P, cfg.F], f32, tag="pv")
                        nc.sync.dma_start(pv[:nrows, :],
                                          prev[r0:r0 + nrows, :])
                        nc.vector.scalar_tensor_tensor(
                            ev[:nrows, :], pt[:nrows, :], 2.0, pv[:nrows, :],
                            op0=mul, op1=sub)
                    nc.scalar.dma_start(dst[r0:r0 + nrows, :], ev[:nrows, :])
                    if bA is not None:
                        evb = evpool.tile([P, cfg.F], bf16, tag="evb")
                        nc.vector.tensor_copy(evb[:nrows, :], ev[:nrows, :])
                        if b < cfg.HBLK:
                            tgt, tr0 = bA, b * P
                        else:
                            tgt, tr0 = bB, (b - cfg.HBLK) * P
                        nc.scalar.dma_start(tgt[tr0:tr0 + nrows, :],
                                            evb[:nrows, :])
                gi += len(grp)
        if s in hooks:
            hooks[s]()


def make_inputs(cfg, meta, per_core, x, W, b):
    import ml_dtypes
    x = np.asarray(x, dtype=np.float32)
    W = np.asarray(W, dtype=np.float32)
    b = np.asarray(b, dtype=np.float32)
    # w_lhsT[f, k, o] = W[o, f*K + k]
    wl = W.reshape(cfg.F, cfg.F, cfg.K).transpose(1, 2, 0)  # W[o, f, k] -> [f,k,o]
    wl = np.ascontiguousarray(wl).reshape(cfg.F, cfg.K * cfg.F)
    xb = x.astype(ml_dtypes.bfloat16)
    x_tabA, x_tabB = cfg.perm_halves(xb)
    x_tabA = np.ascontiguousarray(x_tabA)
    x_tabB = np.ascontiguousarray(x_tabB)
    in_maps = []
    for cid in range(cfg.CORES):
        in_maps.append({
            "x_tabA": x_tabA,
            "x_tabB": x_tabB,
            "x_shard": np.ascontiguousarray(
                x[cid * cfg.RPC:(cid + 1) * cfg.RPC]),
            "idx_all": per_core[cid]["idx_all"],
            "dv_all": per_core[cid]["dv_all"],
            "w_lhsT": wl,
            "b_row": b.reshape(1, cfg.F),
        })
    return in_maps


def kernel(x, lap_rows, lap_cols, lap_vals, W, b, k):
    cfg = Cfg()
    assert int(k) == cfg.K
    meta, per_core = preprocess(cfg, lap_rows, lap_cols, lap_vals)
    nc = build(cfg, meta)
    in_maps = make_inputs(cfg, meta, per_core, x, W, b)
    res = bass_utils.run_bass_kernel_spmd(
        nc, in_maps, core_ids=list(range(cfg.CORES)))
    out = np.concatenate([res.results[c]["out_shard"]
                          for c in range(cfg.CORES)], axis=0)
    return out.astype(np.float32)
